# revision 8
# baseline (speedup 1.0000x reference)
"""GPT-J attention (B=1, S=2048, D=4096, H=16, HD=256, rot=64) on 8 TRN2 cores.

Tensor-parallel over heads (2 heads/core); Wq/Wk/Wv column-sharded, Wo
row-sharded; chunked ReduceScatter of partial outputs; host reassembles.

v2 restructure vs the 878us baseline:
  - phases interleaved: P(s-half0) -> A(qg0,qg1) -> O(qg0,qg1) -> P(half1)
    -> A(qg2,qg3) -> O(qg2,qg3), so the ReduceScatter starts ~250us earlier
    and out-proj hides each attention group's normalization chain.
  - q stays SBUF-resident (no DRAM roundtrip).
  - softmax denominator accumulated on DVE (f32) + one ones-matmul per
    (qg,h) instead of a PE matmul per k-block; reciprocal once per qg with
    both heads stacked at PSUM partitions 0/32.
  - LDWEIGHTS amortized: consecutive matmuls that reuse the stationary
    operand set InstMatmult.ldweights=False (QKV sc-pairs, out-proj fg runs).
  - out-proj loops reordered (stationary ctx^T reused across 4 fg columns,
    4+4 PSUM bank ping-pong), ReduceScatter fired per 256 output rows.

Matmul operands bf16, fp32 PSUM accumulation; masks/softmax sums in fp32.
"""

import contextlib
import numpy as np
import ml_dtypes

import concourse.bass as bass
import concourse.tile as tile
import concourse.mybir as mybir
from concourse import bacc
from concourse.bass_utils import run_bass_kernel_spmd

B, S, D = 1, 2048, 4096
H, HD, ROT = 16, 256, 64
NCORES = 8
HL = H // NCORES          # heads per core = 2
EL = D // NCORES          # local e width = 512
HALFW = S // 2            # 1024
P = 128
NROT2 = ROT // 2          # 32

USE_LDW_SKIP = True

f32 = mybir.dt.float32
bf16 = mybir.dt.bfloat16
EXP = mybir.ActivationFunctionType.Exp
COPY = mybir.ActivationFunctionType.Copy
ADD = mybir.AluOpType.add
MUL = mybir.AluOpType.mult
SUB = mybir.AluOpType.subtract

_CACHE = {}


def _emit(nc, t):
    with tile.TileContext(nc) as tc:
        with contextlib.ExitStack() as _stk:
            ec = _stk.enter_context
            const_pool = ec(tc.tile_pool(name="const", bufs=1))
            wpan_pool = ec(tc.tile_pool(name="wpan", bufs=4))
            stage_pool = ec(tc.tile_pool(name="stage", bufs=8))
            hst_pool = ec(tc.tile_pool(name="hst", bufs=32))
            kres_pool = ec(tc.tile_pool(name="kres", bufs=16))
            vres_pool = ec(tc.tile_pool(name="vres", bufs=16))
            qres_pool = ec(tc.tile_pool(name="qres", bufs=16))
            wot_pool = ec(tc.tile_pool(name="wot", bufs=4))
            rot_pool = ec(tc.tile_pool(name="rot_scr", bufs=1))
            pt_pool = ec(tc.tile_pool(name="pt", bufs=4))
            sacc_pool = ec(tc.tile_pool(name="sacc", bufs=2))
            saccb_pool = ec(tc.tile_pool(name="saccb", bufs=2))
            bbsb_pool = ec(tc.tile_pool(name="bbsb", bufs=2))
            cstg_pool = ec(tc.tile_pool(name="cstg", bufs=8))
            rcp_pool = ec(tc.tile_pool(name="rcp", bufs=2))

            # ---- persistent SBUF state ----
            kres = [[None] * 4 for _ in range(4)]   # [et][scg] -> [128,512]
            vres = [None] * 16                      # [kb]      -> [128,512]
            qres = [[None] * 4 for _ in range(4)]   # [et][scg] -> [128,512]
            cst_store = [[None] * 4 for _ in range(4)]  # [qg][et]

            # half0 hsT loads first so the first matmul starts ASAP
            hst0 = []
            for dt in range(32):
                ht = hst_pool.tile([P, HALFW], bf16, tag="hst", name="hst")
                nc.sync.dma_start(out=ht[:], in_=t["hsT"][dt * P:(dt + 1) * P,
                                                          0:HALFW])
                hst0.append(ht)

            ones_col = const_pool.tile([P, 1], bf16)
            nc.vector.memset(ones_col[:], 1.0)
            ones_row = const_pool.tile([1, P], bf16)
            nc.vector.memset(ones_row[:], 1.0)
            cos_sb = const_pool.tile([NROT2, S], bf16)
            nc.sync.dma_start(out=cos_sb[:], in_=t["cosT"][:])
            sin_sb = const_pool.tile([NROT2, S], bf16)
            nc.sync.dma_start(out=sin_sb[:], in_=t["sinT"][:])
            mask_sb = const_pool.tile([P, 4, 512], bf16)
            nc.sync.dma_start(out=mask_sb[:], in_=t["masks"][:])

            # resident Wo^T (scalar queue; overlaps phase P)
            wot = []
            for et in range(4):
                wtile = wot_pool.tile([P, D], bf16, tag="wot", name="wot")
                nc.scalar.dma_start(out=wtile[:],
                                    in_=t["woT"][et * P:(et + 1) * P, :])
                wot.append(wtile)

            def rot_evict(ps, stg, cols):
                # partitions [0:32) even pairs, [32:64) odd pairs, rest plain
                ca = cos_sb[:, cols:cols + 512]
                sa = sin_sb[:, cols:cols + 512]
                s1 = rot_pool.tile([NROT2, 512], f32, tag="rs1", name="rs1")
                s2 = rot_pool.tile([NROT2, 512], f32, tag="rs2", name="rs2")
                nc.vector.tensor_tensor(s1[:], ps[0:NROT2, :], ca, MUL)
                nc.vector.tensor_tensor(s2[:], ps[NROT2:ROT, :], sa, MUL)
                nc.vector.tensor_tensor(stg[0:NROT2, :], s1[:], s2[:], SUB)
                s3 = rot_pool.tile([NROT2, 512], f32, tag="rs1", name="rs1")
                s4 = rot_pool.tile([NROT2, 512], f32, tag="rs2", name="rs2")
                nc.vector.tensor_tensor(s3[:], ps[NROT2:ROT, :], ca, MUL)
                nc.vector.tensor_tensor(s4[:], ps[0:NROT2, :], sa, MUL)
                nc.vector.tensor_tensor(stg[NROT2:ROT, :], s3[:], s4[:], ADD)
                nc.scalar.activation(stg[ROT:P, :], ps[ROT:P, :], COPY)

            def evict(dst_ap, src_ps, on_act):
                if on_act:
                    nc.scalar.activation(dst_ap, src_ps, COPY)
                else:
                    nc.vector.tensor_copy(dst_ap, src_ps)

            # ================= Phase P: QKV projections for one s-half ======
            def emit_P(half, hst):
                c0 = half * HALFW
                with tc.tile_pool(name=f"psum_p{half}", bufs=8,
                                  space="PSUM") as psum_p:
                    for wname, dst, dst_pool in (("wqT", qres, qres_pool),
                                                 ("wkT", kres, kres_pool)):
                        for wave in range(2):
                            wacc = [psum_p.tile([P, 512], f32, tag="pp",
                                                name="pp") for _ in range(4)]
                            for dt in range(32):
                                wp = wpan_pool.tile([P, HD], bf16, tag="wp",
                                                    name="wp")
                                nc.gpsimd.dma_start(
                                    out=wp[:],
                                    in_=t[wname][dt * P:(dt + 1) * P,
                                                 wave * HD:(wave + 1) * HD])
                                for el in range(2):
                                    for sc in range(2):
                                        mm = nc.tensor.matmul(
                                            wacc[el * 2 + sc][:],
                                            wp[:, el * P:(el + 1) * P],
                                            hst[dt][:, sc * 512:(sc + 1) * 512],
                                            start=(dt == 0), stop=(dt == 31))
                                        # sc==1 reuses the el stationary; keep
                                        # the dt==0 LDW so bank-WAR waits stay
                                        # on a self-loading matmul
                                        if USE_LDW_SKIP and sc == 1 and dt > 0:
                                            mm.ins.ldweights = False
                            for el in range(2):
                                et = wave * 2 + el
                                for sc in range(2):
                                    scg = half * 2 + sc
                                    ps = wacc[el * 2 + sc]
                                    tl = dst_pool.tile([P, 512], bf16,
                                                       tag="r", name="r")
                                    if et in (0, 2):
                                        rot_evict(ps, tl, c0 + sc * 512)
                                    else:
                                        evict(tl[:], ps[:], on_act=(sc == 1))
                                    dst[et][scg] = tl

                    # ---- V: natural layout [s, e] ----
                    acc = [psum_p.tile([P, 512], f32, tag="pp", name="pp")
                           for _ in range(8)]
                    for dt in range(32):
                        wp = wpan_pool.tile([P, EL], bf16, tag="wp", name="wp")
                        nc.gpsimd.dma_start(
                            out=wp[:], in_=t["wvT"][dt * P:(dt + 1) * P, :])
                        for st in range(8):
                            nc.tensor.matmul(
                                acc[st][:],
                                hst[dt][:, st * P:(st + 1) * P],
                                wp[:],
                                start=(dt == 0), stop=(dt == 31))
                    for st in range(8):
                        vtile = vres_pool.tile([P, 512], bf16, tag="vres",
                                               name="vres")
                        evict(vtile[:], acc[st][:], on_act=(st % 2 == 1))
                        vres[half * 8 + st] = vtile

            # ================= Phase A: attention for one q-group ===========
            def emit_A(qg, psum_s, psum_c, psum_ub):
                nkb = 4 * qg + 4
                sump_t = psum_ub.tile([33, 512], f32, tag="pu", name="pu")
                for h in range(HL):
                    e0 = h * HD
                    qt = [qres[h * 2 + 0][qg], qres[h * 2 + 1][qg]]
                    ctxp = [psum_c.tile([P, 512], f32, tag="pc", name="pc")
                            for _ in range(2)]
                    sacc = sacc_pool.tile([P, 512], f32, tag="sa", name="sa")
                    ps_list = [None] * nkb

                    def emit_scores(kb):
                        ps = psum_s.tile([P, 512], f32, tag="ps", name="ps")
                        for half in range(2):
                            nc.tensor.matmul(
                                ps[:],
                                kres[h * 2 + half][kb // 4][
                                    :, (kb % 4) * P:(kb % 4 + 1) * P],
                                qt[half][:],
                                start=(half == 0), stop=(half == 1))
                        ps_list[kb] = ps

                    emit_scores(0)
                    for kb in range(nkb):
                        ps = ps_list[kb]
                        if kb >= 4 * qg:
                            m = kb - 4 * qg
                            nc.vector.tensor_tensor(
                                ps[:], ps[:], mask_sb[:, m, :], ADD)
                        p = pt_pool.tile([P, 512], bf16, tag="pt", name="pt")
                        nc.scalar.activation(p[:], ps[:], EXP)
                        if kb + 1 < nkb:
                            emit_scores(kb + 1)
                        if kb == 0:
                            nc.vector.tensor_copy(sacc[:], p[:])
                        else:
                            nc.vector.tensor_tensor(sacc[:], sacc[:], p[:],
                                                    ADD)
                        for half in range(2):
                            nc.tensor.matmul(
                                ctxp[half][:],
                                vres[kb][:, e0 + half * P:e0 + (half + 1) * P],
                                p[:],
                                start=(kb == 0), stop=(kb == nkb - 1))

                    # head's exp-sum: cast + partition-reduce into row h*32
                    sacc_b = saccb_pool.tile([P, 512], bf16, tag="sb",
                                             name="sb")
                    nc.scalar.activation(sacc_b[:], sacc[:], COPY)
                    nc.tensor.matmul(sump_t[h * 32:h * 32 + 1, :],
                                     ones_col[:], sacc_b[:],
                                     start=True, stop=True)
                    _ctxp_keep[h] = ctxp

                # one reciprocal for both heads (rows 0 and 32)
                recip = rcp_pool.tile([33, 512], f32, tag="rcp", name="rcp")
                nc.vector.reciprocal(recip[:], sump_t[:])
                for h in range(HL):
                    bb = psum_ub.tile([P, 512], f32, tag="pb", name="pb")
                    rb = saccb_pool.tile([1, 512], bf16, tag="rb", name="rb")
                    nc.vector.tensor_copy(rb[:], recip[h * 32:h * 32 + 1, :])
                    nc.tensor.matmul(bb[:], ones_row[:], rb[:],
                                     start=True, stop=True)
                    bb_sb = bbsb_pool.tile([P, 512], bf16, tag="bbsb",
                                           name="bbsb")
                    nc.scalar.activation(bb_sb[:], bb[:], COPY)
                    for half in range(2):
                        cst = cstg_pool.tile([P, 512], bf16, tag="cstg",
                                             name="cstg")
                        nc.vector.tensor_tensor(cst[:],
                                                _ctxp_keep[h][half][:],
                                                bb_sb[:], MUL)
                        cst_store[qg][h * 2 + half] = cst

            _ctxp_keep = [None, None]

            # ============ Phase O: out-proj + chunked ReduceScatter =========
            def emit_O(qg, psum_o):
                cst = cst_store[qg]
                for st in range(4):
                    r0 = qg * 512 + st * P
                    for fh in range(2):
                        po = [psum_o.tile([P, 512], f32, tag="po", name="po")
                              for _ in range(4)]
                        for et in range(4):
                            for fi in range(4):
                                fg = fh * 4 + fi
                                mm = nc.tensor.matmul(
                                    po[fi][:],
                                    cst[et][:, st * P:(st + 1) * P],
                                    wot[et][:, fg * 512:(fg + 1) * 512],
                                    start=(et == 0), stop=(et == 3))
                                # fi>0 reuses the et stationary; keep et==0
                                # LDWs so first bank writes self-load
                                if USE_LDW_SKIP and fi > 0 and et > 0:
                                    mm.ins.ldweights = False
                        for fi in range(4):
                            fg = fh * 4 + fi
                            stg = stage_pool.tile([P, 512], bf16, tag="stg",
                                                  name="stg")
                            evict(stg[:], po[fi][:], on_act=(fg % 2 == 1))
                            nc.sync.dma_start(
                                out=t["pout_d"][r0:r0 + P,
                                                fg * 512:(fg + 1) * 512],
                                in_=stg[:])
                    if st % 2 == 1:
                        start_r = qg * 512 + (st - 1) * P
                        o0 = start_r // NCORES
                        nc.gpsimd.collective_compute(
                            "ReduceScatter",
                            ADD,
                            replica_groups=[list(range(NCORES))],
                            ins=[t["pout_d"][start_r:start_r + 256, :]],
                            outs=[t["rs_d"][o0:o0 + 32, :]],
                        )
                        nc.sync.dma_start(
                            out=t["out"][o0:o0 + 32, :],
                            in_=t["rs_d"][o0:o0 + 32, :])

            # ========================= schedule =============================
            emit_P(0, hst0)

            # half1 hsT loads sit early on the sync queue; WAR deps gate them
            hst1 = []
            for dt in range(32):
                ht = hst_pool.tile([P, HALFW], bf16, tag="hst", name="hst")
                nc.sync.dma_start(out=ht[:], in_=t["hsT"][dt * P:(dt + 1) * P,
                                                          HALFW:S])
                hst1.append(ht)

            with contextlib.ExitStack() as _a1:
                ps_s = _a1.enter_context(
                    tc.tile_pool(name="ps_s1", bufs=2, space="PSUM"))
                ps_c = _a1.enter_context(
                    tc.tile_pool(name="ps_c1", bufs=4, space="PSUM"))
                ps_ub = _a1.enter_context(
                    tc.tile_pool(name="ps_ub1", bufs=1, space="PSUM"))
                emit_A(0, ps_s, ps_c, ps_ub)
                emit_A(1, ps_s, ps_c, ps_ub)
            with tc.tile_pool(name="ps_o1", bufs=8, space="PSUM") as ps_o:
                emit_O(0, ps_o)
                emit_O(1, ps_o)

            emit_P(1, hst1)

            with contextlib.ExitStack() as _a2:
                ps_s = _a2.enter_context(
                    tc.tile_pool(name="ps_s2", bufs=2, space="PSUM"))
                ps_c = _a2.enter_context(
                    tc.tile_pool(name="ps_c2", bufs=4, space="PSUM"))
                ps_ub = _a2.enter_context(
                    tc.tile_pool(name="ps_ub2", bufs=1, space="PSUM"))
                emit_A(2, ps_s, ps_c, ps_ub)
                emit_A(3, ps_s, ps_c, ps_ub)
            with tc.tile_pool(name="ps_o2", bufs=8, space="PSUM") as ps_o:
                emit_O(2, ps_o)
                emit_O(3, ps_o)


def _build():
    if "nc" in _CACHE:
        return _CACHE["nc"]
    nc = bacc.Bacc(None, num_devices=NCORES)
    t = {}
    t["hsT"] = nc.declare_dram_parameter("hsT", [D, S], bf16, isOutput=False)
    t["wqT"] = nc.declare_dram_parameter("wqT", [D, EL], bf16, isOutput=False)
    t["wkT"] = nc.declare_dram_parameter("wkT", [D, EL], bf16, isOutput=False)
    t["wvT"] = nc.declare_dram_parameter("wvT", [D, EL], bf16, isOutput=False)
    t["woT"] = nc.declare_dram_parameter("woT", [EL, D], bf16, isOutput=False)
    t["cosT"] = nc.declare_dram_parameter("cosT", [NROT2, S], bf16, isOutput=False)
    t["sinT"] = nc.declare_dram_parameter("sinT", [NROT2, S], bf16, isOutput=False)
    t["masks"] = nc.declare_dram_parameter("masks", [P, 4, 512], bf16, isOutput=False)
    t["out"] = nc.declare_dram_parameter("out", [S // NCORES, D], bf16, isOutput=True)
    t["pout_d"] = nc.dram_tensor("pout_d", [S, D], bf16)
    t["rs_d"] = nc.dram_tensor("rs_d", [S // NCORES, D], bf16)
    _emit(nc, t)
    nc.compile()
    _CACHE["nc"] = nc
    return nc


def _prep_inputs(hidden_states, Wq, Wk, Wv, Wo, attention_mask, position_ids):
    hs = np.asarray(hidden_states, np.float32).reshape(S, D)
    hsT = np.ascontiguousarray(hs.T).astype(ml_dtypes.bfloat16)

    pos = np.asarray(position_ids).reshape(S).astype(np.float32)
    inv = 10000.0 ** (-np.arange(0, ROT, 2, dtype=np.float32) / ROT)  # [32]
    ang = pos[:, None] * inv[None, :]                                  # [S, 32]
    cosT = np.ascontiguousarray(np.cos(ang).T).astype(ml_dtypes.bfloat16)
    sinT = np.ascontiguousarray(np.sin(ang).T).astype(ml_dtypes.bfloat16)

    am = np.maximum(np.asarray(attention_mask, np.float32).reshape(S, S),
                    np.float32(-1e30))
    masks = np.empty((P, 4, 512), ml_dtypes.bfloat16)
    for m in range(4):
        # transposed-score layout: mask[p, n] for k = m*128+p, q = n
        masks[:, m, :] = am[0:512, m * P:(m + 1) * P].T

    # within-head row permutation: even rot dims, odd rot dims, the rest
    perm1 = np.concatenate([np.arange(0, ROT, 2), np.arange(1, ROT, 2),
                            np.arange(ROT, HD)])
    perm = np.concatenate([perm1 + HD * j for j in range(HL)])

    Wq = np.asarray(Wq, np.float32)
    Wk = np.asarray(Wk, np.float32)
    Wv = np.asarray(Wv, np.float32)
    Wo = np.asarray(Wo, np.float32)
    scale = 1.0 / np.sqrt(np.float32(HD))

    in_maps = []
    for c in range(NCORES):
        rows = slice(c * EL, (c + 1) * EL)
        wq_c = Wq[rows][perm] * scale
        wk_c = Wk[rows][perm]
        wv_c = Wv[rows]
        in_maps.append({
            "hsT": hsT,
            "wqT": np.ascontiguousarray(wq_c.T).astype(ml_dtypes.bfloat16),
            "wkT": np.ascontiguousarray(wk_c.T).astype(ml_dtypes.bfloat16),
            "wvT": np.ascontiguousarray(wv_c.T).astype(ml_dtypes.bfloat16),
            "woT": np.ascontiguousarray(Wo[:, rows].T).astype(ml_dtypes.bfloat16),
            "cosT": cosT,
            "sinT": sinT,
            "masks": masks,
        })
    return in_maps


def run(inputs, trace=False):
    """Run on HW. Returns (full_output, BassKernelResults)."""
    nc = _build()
    in_maps = _prep_inputs(**inputs)
    res = run_bass_kernel_spmd(nc, in_maps, list(range(NCORES)), trace=trace)
    # RS chunk k covers rows [256k, 256k+256); core c holds rows
    # [256k + 32c, +32), stored at rs offset 32k.
    full = np.empty((S, D), np.float32)
    for c in range(NCORES):
        shard = np.asarray(res.results[c]["out"]).astype(np.float32)
        for k in range(8):
            full[256 * k + 32 * c:256 * k + 32 * (c + 1)] = \
                shard[32 * k:32 * (k + 1)]
    return full.reshape(B, S, D), res


def kernel(**inputs):
    full, _ = run(inputs, trace=False)
    return full
